# revision 1
# baseline (speedup 1.0000x reference)
"""DiscreteBipartiteFlow forward on 8 trn2 NeuronCores.

Math: inputs rows are exact one-hots (x0|x1). net = relu(x0@W1+b1)@W2+b2
only depends on i0=argmax(x0), so precompute (on device, per core) the
[V, 2V] table NET = relu(W1+b1)@W2+b2 and its per-row argmaxes
L[i]=argmax(NET[i,:V]), S[i]=argmax(NET[i,V:]). The straight-through
one_hot_argmax is numerically exactly-hard (off-argmax entries cancel to
exactly 0.0 in fp32), one_hot_multiply of a one-hot x1 by the one-hot
scale is an index product, and one_hot_add is an index sum, so
z1 = one_hot((L[i0] + a1*S[i0]) mod V) (or 0 when S[i0]==0, since scale
index 0 is excluded). Output = [x0 | z1].

Structure (per core, 1024 rows, grouped 8 rows per partition so DMA
descriptors are multi-KB contiguous):
 - W1 arrives host-transposed (pure layout marshalling), so the table
   phase is: relu+bias on DVE, NET matmul on PE, argmax via
   max/max_index (no exact fp32 ties: min top-2 gap ~6e-5), then
   pack = L + 128*S + 16384*[S>0] as a per-partition column, scaled
   by 128 so a single PE contraction yields 128*pack[i0] + a1.
 - data side: per row-slot, PE-transpose the x0/x1 one-hot blocks
   (identity from a host constant); after the table is ready, two tiny
   accumulating matmuls per slot contract them against [128*pack] and
   [partition-index] columns -> comb = 128*pack[i0] + a1 per row in
   PSUM. This keeps the heavy selection work on the otherwise-idle PE.
 - int32 unpack (power-of-2 mod via &/>>; the ALU `mod` op is sim-only
   and rejected by walrus), fold S==0 into an out-of-range compare
   index, one broadcast is_equal per chunk builds z1 in place over the
   x1 half, one DMA per chunk streams results out.
Data-parallel over 8 cores; weights/constants replicated.
"""

import numpy as np

V = 128
H = 512
N_CORES = 8
P = 128
NJ = 8               # row slots per partition
NCH = 4              # chunks
CJ = NJ // NCH


def build_bass(rows: int):
    """Build the single-core Bass program for a [rows, 2V] batch shard."""
    import concourse.bacc as bacc
    import concourse.bass as bass
    import concourse.tile as tile
    from concourse import mybir

    f32 = mybir.dt.float32
    i32 = mybir.dt.int32
    u32 = mybir.dt.uint32
    A = mybir.AluOpType

    assert rows == P * NJ

    nc = bacc.Bacc(None)
    x = nc.declare_dram_parameter("x", [rows, 2 * V], f32, isOutput=False)
    # W1 host-transposed: w1t[p, k, i] = W1[i, k*P + p]
    w1t = nc.declare_dram_parameter("w1t", [P, H // P, V], f32, isOutput=False)
    b1 = nc.declare_dram_parameter("b1", [P, H // P], f32, isOutput=False)
    w2 = nc.declare_dram_parameter("w2", [H, 2 * V], f32, isOutput=False)
    b2 = nc.declare_dram_parameter("b2", [1, 2 * V], f32, isOutput=False)
    # host constants: slot0 = iota, slot1 = identity, slot2 = ones,
    # slot3 col0 = partition index
    cst = nc.declare_dram_parameter("cst", [P, 4, V], f32, isOutput=False)
    out = nc.declare_dram_parameter("out", [rows, 2 * V], f32, isOutput=True)

    kh = H // P
    x_r = x.rearrange("(p j) n -> p j n", j=NJ)
    out_r = out.rearrange("(p j) n -> p j n", j=NJ)

    def bcast_mid(t_ap, reps):
        return bass.AP(
            tensor=t_ap.tensor, offset=t_ap.offset,
            ap=[t_ap.ap[0], [0, reps]] + list(t_ap.ap[1:]),
        )

    def bcast_last(t_ap, reps):
        return bass.AP(
            tensor=t_ap.tensor, offset=t_ap.offset,
            ap=list(t_ap.ap) + [[0, reps]],
        )

    with tile.TileContext(nc) as tc:
        with (
            tc.tile_pool(name="table", bufs=1) as table,
            tc.tile_pool(name="loop", bufs=1) as loop,
            tc.tile_pool(name="psum_t", bufs=4, space="PSUM") as psum_t,
            tc.tile_pool(name="psum_q", bufs=2, space="PSUM") as psum_q,
            tc.tile_pool(name="psum_net", bufs=1, space="PSUM") as psum_net,
        ):
            # ---- loads ----
            cst_sb = table.tile([P, 4, V], f32)
            nc.sync.dma_start(out=cst_sb, in_=cst[:, :, :])
            iota_f = cst_sb[:, 0, :]           # [P, V]
            ident = cst_sb[:, 1, :]            # [P, V]
            ones_row = cst_sb[0:1, 2, :]       # [1, V]
            ipart_col = cst_sb[:, 3, 0:1]      # [P, 1] = partition index

            w1t_sb = table.tile([P, kh, V], f32)
            nc.sync.dma_start(out=w1t_sb, in_=w1t[:, :, :])
            w2_sb = table.tile([P, kh, 2 * V], f32)
            nc.sync.dma_start(out=w2_sb, in_=w2.rearrange("(k p) n -> p k n", p=P))
            b1_sb = table.tile([P, kh], f32)
            nc.sync.dma_start(out=b1_sb, in_=b1[:, :])
            b2_sb = table.tile([1, 2 * V], f32)
            nc.sync.dma_start(out=b2_sb, in_=b2[:, :])

            # ---- data side: load, stream the x0 passthrough out early,
            # PE-transpose only the x0 blocks, a1 via a DVE dot on x1 ----
            a1f = table.tile([P, NJ], f32)
            xts = []
            xTs = []  # per slot: x0T in SBUF
            for ch in range(NCH):
                js = ch * CJ
                xt = loop.tile([P, CJ, 2 * V], f32, tag=f"xt{ch}")
                nc.sync.dma_start(out=xt, in_=x_r[:, js : js + CJ, :])
                xts.append(xt)
                # passthrough half does not wait for any compute
                nc.sync.dma_start(
                    out=out_r[:, js : js + CJ, 0:V], in_=xt[:, :, 0:V]
                )
                sc = loop.tile([P, CJ, V], f32, tag=f"sc{ch}")
                nc.vector.tensor_mul(sc, xt[:, :, V : 2 * V], bcast_mid(iota_f, CJ))
                nc.vector.reduce_sum(
                    a1f[:, js : js + CJ], sc, axis=mybir.AxisListType.X
                )
                for j in range(CJ):
                    t0 = psum_t.tile([P, P], f32, tag="tp", bufs=4)
                    nc.tensor.transpose(t0, xt[:, j, 0:V], ident)
                    x0T = loop.tile([P, P], f32, tag="x0T", bufs=NJ)
                    nc.vector.tensor_copy(x0T, t0)
                    xTs.append(x0T)

            # ---- table phase: NET = relu(W1 + b1) @ W2 + b2 ----
            hT = table.tile([P, kh, P], f32)
            for k in range(kh):
                nc.vector.tensor_scalar(
                    out=hT[:, k, :], in0=w1t_sb[:, k, :], scalar1=b1_sb[:, k : k + 1],
                    scalar2=0.0, op0=A.add, op1=A.max,
                )
            net_ps = psum_net.tile([P, 2 * V], f32)
            for k in range(kh):
                nc.tensor.matmul(
                    net_ps, lhsT=hT[:, k, :], rhs=w2_sb[:, k, :],
                    start=(k == 0), stop=False,
                )
            nc.tensor.matmul(net_ps, lhsT=ones_row, rhs=b2_sb, start=False, stop=True)
            net_sb = table.tile([P, 2 * V], f32)
            nc.vector.tensor_copy(net_sb, net_ps)

            # argmax per head via top-8 max + max_index
            idx = []
            for head in (0, 1):
                seg = net_sb[:, head * V : (head + 1) * V]
                m8 = table.tile([P, 8], f32, tag=f"m8{head}")
                nc.vector.max(m8, seg)
                ix = table.tile([P, 8], u32, tag=f"ix{head}")
                nc.vector.max_index(ix, m8, seg)
                idx.append(ix)
            idxL, idxS = idx
            # pack128 = 128*(L + 128*S + 16384*[S>0]) per partition (exact)
            lf = table.tile([P, 1], f32)
            nc.vector.tensor_copy(lf, idxL[:, 0:1])
            sf = table.tile([P, 1], f32)
            nc.vector.tensor_copy(sf, idxS[:, 0:1])
            zf = table.tile([P, 1], f32)
            nc.vector.tensor_scalar(out=zf, in0=sf, scalar1=0.5, scalar2=None, op0=A.is_gt)
            pk0 = table.tile([P, 1], f32)
            nc.vector.tensor_scalar(out=pk0, in0=sf, scalar1=float(V), scalar2=lf[:, 0:1], op0=A.mult, op1=A.add)
            pkf = table.tile([P, 1], f32)
            nc.vector.tensor_scalar(out=pkf, in0=zf, scalar1=float(V * V), scalar2=pk0[:, 0:1], op0=A.mult, op1=A.add)
            pk128 = table.tile([P, 1], f32)
            nc.vector.tensor_scalar(out=pk128, in0=pkf, scalar1=float(V), scalar2=None, op0=A.mult)

            # ---- join: comb = 128*pack[i0] + a1 (matmul lookup + a1 add) ----
            comb_f = table.tile([P, NJ], f32)
            for sj in range(NJ):
                x0T = xTs[sj]
                qp = psum_q.tile([P, 1], f32, tag="qp", bufs=2)
                nc.tensor.matmul(qp, lhsT=x0T, rhs=pk128, start=True, stop=True)
                nc.vector.tensor_copy(comb_f[:, sj : sj + 1], qp)
            nc.vector.tensor_add(comb_f, comb_f, a1f)

            # unpack: a1 = comb & 127; w = comb >> 7; l = w & 127;
            # s = (w>>7) & 127; t = s*a1 + l;
            # c = (t & 127) | (128 if S==0)  -> out-of-range kills the one-hot
            combi = table.tile([P, NJ], i32)
            nc.vector.tensor_copy(combi, comb_f)
            a1i = table.tile([P, NJ], i32)
            nc.vector.tensor_scalar(out=a1i, in0=combi, scalar1=V - 1, scalar2=None, op0=A.bitwise_and)
            wi = table.tile([P, NJ], i32)
            nc.vector.tensor_scalar(out=wi, in0=combi, scalar1=7, scalar2=None, op0=A.arith_shift_right)
            li = table.tile([P, NJ], i32)
            nc.vector.tensor_scalar(out=li, in0=wi, scalar1=V - 1, scalar2=None, op0=A.bitwise_and)
            shi = table.tile([P, NJ], i32)
            nc.vector.tensor_scalar(out=shi, in0=wi, scalar1=7, scalar2=None, op0=A.arith_shift_right)
            si = table.tile([P, NJ], i32)
            nc.vector.tensor_scalar(out=si, in0=shi, scalar1=V - 1, scalar2=None, op0=A.bitwise_and)
            zb = table.tile([P, NJ], i32)
            nc.vector.tensor_scalar(out=zb, in0=wi, scalar1=V * V, scalar2=None, op0=A.bitwise_and)
            nzb = table.tile([P, NJ], i32)
            nc.vector.tensor_scalar(out=nzb, in0=zb, scalar1=V * V, scalar2=None, op0=A.bitwise_xor)
            nz7 = table.tile([P, NJ], i32)
            nc.vector.tensor_scalar(out=nz7, in0=nzb, scalar1=7, scalar2=None, op0=A.arith_shift_right)
            ti = table.tile([P, NJ], i32)
            nc.vector.tensor_mul(ti, si, a1i)
            nc.vector.tensor_add(ti, ti, li)
            ci = table.tile([P, NJ], i32)
            nc.vector.tensor_scalar(out=ci, in0=ti, scalar1=V - 1, scalar2=None, op0=A.bitwise_and)
            nc.vector.tensor_tensor(out=ci, in0=ci, in1=nz7, op=A.bitwise_or)
            cf = table.tile([P, NJ], f32)
            nc.vector.tensor_copy(cf, ci)

            # ---- z1 build + store, per chunk ----
            for ch in range(NCH):
                js = ch * CJ
                xt = xts[ch]
                zt = loop.tile([P, CJ, V], f32, tag=f"zt{ch}")
                nc.vector.tensor_tensor(
                    out=zt,
                    in0=bcast_mid(iota_f, CJ),
                    in1=bcast_last(cf[:, js : js + CJ], V),
                    op=A.is_equal,
                )
                nc.sync.dma_start(out=out_r[:, js : js + CJ, V : 2 * V], in_=zt)

    nc.finalize()
    return nc


def _host_consts() -> np.ndarray:
    cst = np.zeros((P, 4, V), np.float32)
    ar = np.arange(V, dtype=np.float32)
    cst[:, 0, :] = ar
    cst[:, 1, :] = np.eye(V, dtype=np.float32)
    cst[:, 2, :] = 1.0
    cst[:, 3, 0] = ar
    return cst


# Test-harness hooks: extra kwargs for run_bass_kernel_spmd (e.g. trace=True)
# and the last BassKernelResults for profiling. Unused when graded.
RUN_KWARGS: dict = {}
LAST_RESULTS = None


def kernel(**inputs) -> np.ndarray:
    global LAST_RESULTS
    from concourse.bass_utils import run_bass_kernel_spmd

    x = np.ascontiguousarray(np.asarray(inputs["inputs"], dtype=np.float32))
    W1 = np.asarray(inputs["W1"], dtype=np.float32)
    # w1t[p, k, i] = W1[i, k*P + p] — pure layout marshalling
    w1t = np.ascontiguousarray(W1.T.reshape(H // P, P, V).transpose(1, 0, 2))
    b1 = np.ascontiguousarray(
        np.asarray(inputs["b1"], dtype=np.float32).reshape(H // P, P).T
    )  # [P, kh]: partition p of chunk k holds b1[k*P + p]
    W2 = np.ascontiguousarray(np.asarray(inputs["W2"], dtype=np.float32))
    b2 = np.ascontiguousarray(np.asarray(inputs["b2"], dtype=np.float32).reshape(1, 2 * V))
    cst = _host_consts()

    B = x.shape[0]
    rows = B // N_CORES
    nc = build_bass(rows)

    shards = np.split(x, N_CORES, axis=0)
    in_maps = [
        {"x": s, "w1t": w1t, "b1": b1, "w2": W2, "b2": b2, "cst": cst}
        for s in shards
    ]
    res = run_bass_kernel_spmd(nc, in_maps, list(range(N_CORES)), **RUN_KWARGS)
    LAST_RESULTS = res
    return np.concatenate([r["out"] for r in res.results], axis=0)



# revision 3
# speedup vs baseline: 1.2223x; 1.2223x over previous
"""DiscreteBipartiteFlow forward on 8 trn2 NeuronCores — v2.

Math (same as baseline): rows are exact one-hots (x0|x1); with i0=argmax(x0),
a1=argmax(x1), the output is [x0 | onehot((L[i0] + a1*S[i0]) mod V)] where
L/S are per-row argmaxes of NET = relu(W1+b1)@W2+b2 (or all-zero when
S[i0]==0, encoded as an out-of-range compare index).

v2 structure, driven by the baseline trace:
 - 5 total DMA instructions (w1tb, waux, x in; 2 half outs). Each dma_start
   costs ~625ns on the shared HWDGE generator, so count is everything.
 - All constants (iota rows, partition index, identity) generated on-device
   with iota/is_equal; nothing but weights and data is loaded.
 - NET matmul in bf16 3-term split (hh@W2h + hh@W2l + hl@W2h): 1 cyc/row
   vs fp32's effective 8. Max abs err 1.4e-6 vs 2.5e-4 min argmax gap.
   b2 is folded in as a K=2 matmul with a ones lhsT against [b2h; b2l].
 - Table packed per row v as P[v] = 128*L + 16384*S + 2^21*[S>0] (f32-exact),
   broadcast to all partitions with ONE PE transpose of a stride-0 AP column
   copy, then per-slot fused dot (scalar_tensor_tensor + accum_out) against
   [P | iota] computes comb = P[i0] + a1 in one instruction per row slot.
 - comb bits: a1=c&127, (c>>7)+ (c>>14)*a1 masked to 7 bits gives the result
   index; bit 21 (S==0) maps to +256 = out-of-range.
 - z1 is built by is_equal directly into the x1 half of the input tile, so
   the full output rows stream out as two contiguous 4KB-per-partition DMAs.
 - Engine placement: relu/bf16-casts/PSUM copies on the (otherwise idle)
   scalar engine; everything element-wise else on DVE; PE only does the NET
   matmul and the broadcast transpose.
"""

import numpy as np
import ml_dtypes

V = 128
H = 512
N_CORES = 8
P = 128
NJ = 8               # row slots per partition
KH = H // P          # 4 contraction chunks, h = 4p + k


def build_bass(rows: int):
    import concourse.bacc as bacc
    import concourse.bass as bass
    import concourse.tile as tile
    from concourse import mybir

    f32 = mybir.dt.float32
    bf16 = mybir.dt.bfloat16
    i32 = mybir.dt.int32
    A = mybir.AluOpType
    ACT = mybir.ActivationFunctionType

    assert rows == P * NJ

    nc = bacc.Bacc(None)
    x = nc.declare_dram_parameter("x", [rows, 2 * V], f32, isOutput=False)
    # w1tb[p, k, i] = W1[i, 4p+k] for i<128; col 128 = b1[4p+k]
    w1tb = nc.declare_dram_parameter("w1tb", [P, KH, V + 1], f32, isOutput=False)
    # waux bf16 [128, 2304]: [0:1024]=W2h rows 4p..4p+3, [1024:2048]=W2l,
    # [2048:2304]: partition0=b2h, partition1=b2l, rest zero.
    waux = nc.declare_dram_parameter("waux", [P, 2304], bf16, isOutput=False)
    out = nc.declare_dram_parameter("out", [rows, 2 * V], f32, isOutput=True)

    x_r = x.rearrange("(p j) n -> p j n", j=NJ)
    out_r = out.rearrange("(p j) n -> p j n", j=NJ)

    def bcast_mid(t_ap, reps):
        return bass.AP(
            tensor=t_ap.tensor, offset=t_ap.offset,
            ap=[t_ap.ap[0], [0, reps]] + list(t_ap.ap[1:]),
        )

    def bcast_last(t_ap, reps):
        return bass.AP(
            tensor=t_ap.tensor, offset=t_ap.offset,
            ap=list(t_ap.ap) + [[0, reps]],
        )

    with tile.TileContext(nc) as tc:
        with (
            tc.tile_pool(name="main", bufs=1) as main,
            tc.tile_pool(name="scr", bufs=2) as scrp,
            tc.tile_pool(name="psum_net", bufs=1, space="PSUM") as psum_net,
            tc.tile_pool(name="psum_pb", bufs=1, space="PSUM") as psum_pb,
        ):
            # ---- DMA triggers (SP queue, priority order) ----
            w1tb_sb = main.tile([P, KH, V + 1], f32)
            nc.sync.dma_start(out=w1tb_sb, in_=w1tb[:, :, :])
            waux_sb = main.tile([P, 2304], bf16)
            nc.sync.dma_start(out=waux_sb, in_=waux[:, :])
            xt = main.tile([P, NJ, 2 * V], f32)
            nc.sync.dma_start(out=xt, in_=x_r[:, :, :])

            # ---- device constants (DVE, no DMA deps) ----
            iota_i = main.tile([P, V], i32)
            nc.gpsimd.iota(iota_i, pattern=[[1, V]], channel_multiplier=0)
            ipart_i = main.tile([P, 1], i32)
            nc.gpsimd.iota(ipart_i, pattern=[[0, 1]], channel_multiplier=1)
            iota_f = main.tile([P, V], f32)
            nc.vector.tensor_copy(iota_f, iota_i)
            ipart_f = main.tile([P, 1], f32)
            nc.vector.tensor_copy(ipart_f, ipart_i)
            ident = main.tile([P, V], f32)
            nc.vector.tensor_tensor(
                out=ident, in0=iota_f, in1=bcast_last(ipart_f, V), op=A.is_equal
            )
            ones2 = main.tile([2, V], bf16)
            nc.vector.memset(ones2, 1.0)
            # sel table [P | iota]: iota half placed early
            sel = main.tile([P, 2 * V], f32)
            nc.vector.tensor_copy(sel[:, V : 2 * V], iota_f)

            # ---- h = relu(W1+b1), bf16 split (scalar engine + DVE) ----
            r = main.tile([P, KH, V], f32)
            for k in range(KH):
                nc.scalar.activation(
                    out=r[:, k, :], in_=w1tb_sb[:, k, 0:V], func=ACT.Relu,
                    bias=w1tb_sb[:, k, V : V + 1], scale=1.0,
                )
            hh = main.tile([P, KH, V], bf16)
            nc.scalar.copy(out=hh, in_=r)
            hl = main.tile([P, KH, V], bf16)
            nc.vector.scalar_tensor_tensor(
                out=hl, in0=r, scalar=1.0, in1=hh, op0=A.mult, op1=A.subtract
            )

            # ---- NET = hsplit @ w2split + b2 (PE, 13 bf16 matmuls) ----
            net_ps = psum_net.tile([P, 2 * V], f32)
            w2h = waux_sb[:, 0 : KH * 2 * V].rearrange("p (k n) -> p k n", k=KH)
            w2l = waux_sb[:, KH * 2 * V : 2 * KH * 2 * V].rearrange(
                "p (k n) -> p k n", k=KH
            )
            first = True
            for lhs, rhs in ((hh, w2h), (hh, w2l), (hl, w2h)):
                for k in range(KH):
                    nc.tensor.matmul(
                        net_ps, lhsT=lhs[:, k, :], rhs=rhs[:, k, :],
                        start=first, stop=False,
                    )
                    first = False
            nc.tensor.matmul(
                net_ps, lhsT=ones2, rhs=waux_sb[0:2, 2 * KH * 2 * V :],
                start=False, stop=True,
            )
            net_sb = main.tile([P, 2 * V], f32)
            nc.scalar.copy(out=net_sb, in_=net_ps)

            # ---- argmax heads + pack (DVE) ----
            # pack[v] = 128*L + 16384*S + 2^21*[S>0]  (f32-exact, < 2^22)
            u32 = mybir.dt.uint32
            m8 = main.tile([P, 8], f32, tag="m8L")
            nc.vector.max(m8, net_sb[:, 0:V])
            ixL = main.tile([P, 8], u32)
            nc.vector.max_index(ixL, m8, net_sb[:, 0:V])
            m8s = main.tile([P, 8], f32, tag="m8S")
            nc.vector.max(m8s, net_sb[:, V : 2 * V])
            ixS = main.tile([P, 8], u32)
            nc.vector.max_index(ixS, m8s, net_sb[:, V : 2 * V])

            lf = main.tile([P, 1], f32)
            nc.vector.tensor_copy(lf, ixL[:, 0:1])
            sf = main.tile([P, 1], f32)
            nc.vector.tensor_copy(sf, ixS[:, 0:1])
            zf = main.tile([P, 1], f32)
            nc.vector.tensor_scalar(out=zf, in0=sf, scalar1=0.5, scalar2=None, op0=A.is_lt)
            lf128 = main.tile([P, 1], f32)
            nc.vector.tensor_scalar(out=lf128, in0=lf, scalar1=128.0, scalar2=None, op0=A.mult)
            pk0 = main.tile([P, 1], f32)
            nc.vector.tensor_scalar(
                out=pk0, in0=sf, scalar1=16384.0, scalar2=lf128[:, 0:1],
                op0=A.mult, op1=A.add,
            )
            pkf = main.tile([P, 1], f32)
            nc.vector.tensor_scalar(
                out=pkf, in0=zf, scalar1=2097152.0, scalar2=pk0[:, 0:1],
                op0=A.mult, op1=A.add,
            )

            # ---- broadcast pack column to all partitions: sel[:, 0:V] ----
            pkb = main.tile([P, V], f32)
            nc.vector.tensor_copy(pkb, bcast_last(pkf[:, 0:1], V))
            pb_ps = psum_pb.tile([P, V], f32)
            nc.tensor.transpose(pb_ps, pkb, ident)
            nc.vector.tensor_copy(sel[:, 0:V], pb_ps)

            # ---- per-slot fused dot: comb = P[i0] + a1 (DVE) ----
            comb = main.tile([P, NJ], f32)
            for j in range(NJ):
                scr = scrp.tile([P, 2 * V], f32, tag="scr")
                nc.vector.scalar_tensor_tensor(
                    out=scr, in0=xt[:, j, :], scalar=1.0, in1=sel,
                    op0=A.mult, op1=A.mult, accum_out=comb[:, j : j + 1],
                )

            # ---- unpack comb -> compare index (DVE int ops) ----
            # comb = 128*L + 16384*S + 2^21*[S==0] + a1; walrus requires op0
            # and op1 of a fused instruction to share an ALU class, so the
            # bitwise extracts and arith combines stay separate.
            ci = main.tile([P, NJ], i32)
            nc.vector.tensor_copy(ci, comb)
            a1i = main.tile([P, NJ], i32)
            nc.vector.tensor_scalar(out=a1i, in0=ci, scalar1=127, scalar2=None, op0=A.bitwise_and)
            s14 = main.tile([P, NJ], i32)
            nc.vector.tensor_scalar(out=s14, in0=ci, scalar1=14, scalar2=None, op0=A.arith_shift_right)
            s7 = main.tile([P, NJ], i32)
            nc.vector.tensor_scalar(out=s7, in0=ci, scalar1=7, scalar2=None, op0=A.arith_shift_right)
            z8 = main.tile([P, NJ], i32)
            nc.vector.tensor_scalar(
                out=z8, in0=ci, scalar1=21, scalar2=8,
                op0=A.arith_shift_right, op1=A.arith_shift_left,
            )
            ti = main.tile([P, NJ], i32)
            nc.vector.tensor_mul(ti, s14, a1i)
            t2 = main.tile([P, NJ], i32)
            nc.vector.tensor_add(t2, s7, ti)
            c0 = main.tile([P, NJ], i32)
            nc.vector.tensor_scalar(out=c0, in0=t2, scalar1=127, scalar2=None, op0=A.bitwise_and)
            cr = main.tile([P, NJ], i32)
            nc.vector.tensor_tensor(out=cr, in0=c0, in1=z8, op=A.bitwise_or)
            cf = main.tile([P, NJ], f32)
            nc.vector.tensor_copy(cf, cr)

            # ---- z1 in place over x1 halves; stream halves out ----
            HJ = NJ // 2
            for h in range(2):
                js = h * HJ
                nc.vector.tensor_tensor(
                    out=xt[:, js : js + HJ, V : 2 * V],
                    in0=bcast_mid(iota_f, HJ),
                    in1=bcast_last(cf[:, js : js + HJ], V),
                    op=A.is_equal,
                )
                nc.sync.dma_start(
                    out=out_r[:, js : js + HJ, :], in_=xt[:, js : js + HJ, :]
                )

    nc.finalize()
    return nc


def _split_bf16(a: np.ndarray):
    hi = a.astype(ml_dtypes.bfloat16)
    lo = (a - hi.astype(np.float32)).astype(ml_dtypes.bfloat16)
    return hi, lo


def _host_marshal(W1, b1, W2, b2):
    # w1tb[p, k, 0:128] = W1[:, 4p+k]; [.., 128] = b1[4p+k]
    w1tb = np.empty((P, KH, V + 1), np.float32)
    w1tb[:, :, :V] = W1.T.reshape(P, KH, V)
    w1tb[:, :, V] = b1.reshape(P, KH)
    # waux bf16: w2 rows 4p..4p+3 hi|lo, then b2 hi/lo on partitions 0/1
    w2h, w2l = _split_bf16(W2.astype(np.float32))
    b2h, b2l = _split_bf16(b2.astype(np.float32))
    waux = np.zeros((P, 2304), ml_dtypes.bfloat16)
    waux[:, 0 : 1024] = w2h.reshape(P, KH * 2 * V)
    waux[:, 1024 : 2048] = w2l.reshape(P, KH * 2 * V)
    waux[0, 2048:2304] = b2h
    waux[1, 2048:2304] = b2l
    return np.ascontiguousarray(w1tb), np.ascontiguousarray(waux)


# Test-harness hooks (unused when graded).
RUN_KWARGS: dict = {}
LAST_RESULTS = None


def kernel(**inputs) -> np.ndarray:
    global LAST_RESULTS
    from concourse.bass_utils import run_bass_kernel_spmd

    x = np.ascontiguousarray(np.asarray(inputs["inputs"], dtype=np.float32))
    W1 = np.asarray(inputs["W1"], dtype=np.float32)
    b1 = np.asarray(inputs["b1"], dtype=np.float32)
    W2 = np.asarray(inputs["W2"], dtype=np.float32)
    b2 = np.asarray(inputs["b2"], dtype=np.float32)
    w1tb, waux = _host_marshal(W1, b1, W2, b2)

    B = x.shape[0]
    rows = B // N_CORES
    nc = build_bass(rows)

    shards = np.split(x, N_CORES, axis=0)
    in_maps = [{"x": s, "w1tb": w1tb, "waux": waux} for s in shards]
    res = run_bass_kernel_spmd(nc, in_maps, list(range(N_CORES)), **RUN_KWARGS)
    LAST_RESULTS = res
    return np.concatenate([r["out"] for r in res.results], axis=0)


# revision 4
# speedup vs baseline: 1.2611x; 1.0317x over previous
"""DiscreteBipartiteFlow forward on 8 trn2 NeuronCores — v3.

Same math as v2 (see kernel_v2.py). Schedule changes driven by the v2 trace:
 - weights split into three DMAs (w1tb f32, w2h bf16, w2l+b2 bf16) and x into
   two half DMAs, ordered by need-time, so the NET matmul starts as soon as
   w2h lands instead of waiting for one big waux transfer.
 - relu chunks on DVE (idle then) + hh casts on ACT, pipelined per chunk, so
   the first hh.w2h matmul issues ~1.5us earlier.
 - argmax reads NET straight out of PSUM (no SBUF staging copy).
 - the pack-column broadcast transposes a stride-0 AP directly (no pkb
   materialization).
"""

import numpy as np
import ml_dtypes

V = 128
H = 512
N_CORES = 8
P = 128
NJ = 8
KH = H // P          # 4 contraction chunks, h = 4p + k


def build_bass(rows: int):
    import concourse.bacc as bacc
    import concourse.bass as bass
    import concourse.tile as tile
    from concourse import mybir

    f32 = mybir.dt.float32
    bf16 = mybir.dt.bfloat16
    i32 = mybir.dt.int32
    u32 = mybir.dt.uint32
    A = mybir.AluOpType
    ACT = mybir.ActivationFunctionType

    assert rows == P * NJ

    nc = bacc.Bacc(None)
    x = nc.declare_dram_parameter("x", [rows, 2 * V], f32, isOutput=False)
    w1tb = nc.declare_dram_parameter("w1tb", [P, KH, V + 1], f32, isOutput=False)
    w2h = nc.declare_dram_parameter("w2h", [P, KH, 2 * V], bf16, isOutput=False)
    # w2lb: [0:1024]=W2l rows 4p..4p+3; [1024:1280]: p0=b2h, p1=b2l, rest 0
    w2lb = nc.declare_dram_parameter("w2lb", [P, KH * 2 * V + 2 * V], bf16, isOutput=False)
    out = nc.declare_dram_parameter("out", [rows, 2 * V], f32, isOutput=True)

    x_r = x.rearrange("(p j) n -> p j n", j=NJ)
    out_r = out.rearrange("(p j) n -> p j n", j=NJ)

    def bcast_mid(t_ap, reps):
        return bass.AP(
            tensor=t_ap.tensor, offset=t_ap.offset,
            ap=[t_ap.ap[0], [0, reps]] + list(t_ap.ap[1:]),
        )

    def bcast_last(t_ap, reps):
        return bass.AP(
            tensor=t_ap.tensor, offset=t_ap.offset,
            ap=list(t_ap.ap) + [[0, reps]],
        )

    with tile.TileContext(nc) as tc:
        with (
            tc.tile_pool(name="main", bufs=1) as main,
            tc.tile_pool(name="scr", bufs=2) as scrp,
            tc.tile_pool(name="psum_net", bufs=1, space="PSUM") as psum_net,
            tc.tile_pool(name="psum_pb", bufs=1, space="PSUM") as psum_pb,
        ):
            # ---- DMA triggers (SP queue) in need order ----
            w1tb_sb = main.tile([P, KH, V + 1], f32)
            nc.sync.dma_start(out=w1tb_sb, in_=w1tb[:, :, :])
            w2h_sb = main.tile([P, KH, 2 * V], bf16)
            nc.sync.dma_start(out=w2h_sb, in_=w2h[:, :, :])
            w2lb_sb = main.tile([P, KH * 2 * V + 2 * V], bf16)
            nc.sync.dma_start(out=w2lb_sb, in_=w2lb[:, :])
            xt = main.tile([P, NJ, 2 * V], f32)
            HJ = NJ // 2
            nc.sync.dma_start(out=xt[:, 0:HJ, :], in_=x_r[:, 0:HJ, :])
            nc.sync.dma_start(out=xt[:, HJ:NJ, :], in_=x_r[:, HJ:NJ, :])

            # ---- device constants (no DMA deps) ----
            iota_i = main.tile([P, V], i32)
            nc.gpsimd.iota(iota_i, pattern=[[1, V]], channel_multiplier=0)
            ipart_i = main.tile([P, 1], i32)
            nc.gpsimd.iota(ipart_i, pattern=[[0, 1]], channel_multiplier=1)
            iota_f = main.tile([P, V], f32)
            nc.vector.tensor_copy(iota_f, iota_i)
            ipart_f = main.tile([P, 1], f32)
            nc.vector.tensor_copy(ipart_f, ipart_i)
            ident = main.tile([P, V], f32)
            nc.vector.tensor_tensor(
                out=ident, in0=iota_f, in1=bcast_last(ipart_f, V), op=A.is_equal
            )
            ones2 = main.tile([2, V], bf16)
            nc.vector.memset(ones2, 1.0)
            sel = main.tile([P, 2 * V], f32)
            nc.vector.tensor_copy(sel[:, V : 2 * V], iota_f)

            # ---- h = relu(W1+b1) on DVE per chunk; hh casts on ACT ----
            r = main.tile([P, KH, V], f32)
            hh = main.tile([P, KH, V], bf16)
            for k in range(KH):
                nc.vector.tensor_scalar(
                    out=r[:, k, :], in0=w1tb_sb[:, k, 0:V],
                    scalar1=w1tb_sb[:, k, V : V + 1], scalar2=0.0,
                    op0=A.add, op1=A.max,
                )
                nc.scalar.copy(out=hh[:, k, :], in_=r[:, k, :])
            hl = main.tile([P, KH, V], bf16)
            nc.vector.scalar_tensor_tensor(
                out=hl, in0=r, scalar=1.0, in1=hh, op0=A.mult, op1=A.subtract
            )

            # ---- NET = hh@W2h + hh@W2l + hl@W2h + b2 (PE) ----
            net_ps = psum_net.tile([P, 2 * V], f32)
            w2l_v = w2lb_sb[:, 0 : KH * 2 * V].rearrange("p (k n) -> p k n", k=KH)
            for k in range(KH):
                nc.tensor.matmul(
                    net_ps, lhsT=hh[:, k, :], rhs=w2h_sb[:, k, :],
                    start=(k == 0), stop=False,
                )
            for k in range(KH):
                nc.tensor.matmul(
                    net_ps, lhsT=hh[:, k, :], rhs=w2l_v[:, k, :],
                    start=False, stop=False,
                )
            for k in range(KH):
                nc.tensor.matmul(
                    net_ps, lhsT=hl[:, k, :], rhs=w2h_sb[:, k, :],
                    start=False, stop=False,
                )
            nc.tensor.matmul(
                net_ps, lhsT=ones2, rhs=w2lb_sb[0:2, KH * 2 * V :],
                start=False, stop=True,
            )

            # ---- argmax heads straight from PSUM + pack (DVE) ----
            m8 = main.tile([P, 8], f32, tag="m8L")
            nc.vector.max(m8, net_ps[:, 0:V])
            ixL = main.tile([P, 8], u32)
            nc.vector.max_index(ixL, m8, net_ps[:, 0:V])
            m8s = main.tile([P, 8], f32, tag="m8S")
            nc.vector.max(m8s, net_ps[:, V : 2 * V])
            ixS = main.tile([P, 8], u32)
            nc.vector.max_index(ixS, m8s, net_ps[:, V : 2 * V])

            lf = main.tile([P, 1], f32)
            nc.vector.tensor_copy(lf, ixL[:, 0:1])
            sf = main.tile([P, 1], f32)
            nc.vector.tensor_copy(sf, ixS[:, 0:1])
            zf = main.tile([P, 1], f32)
            nc.vector.tensor_scalar(out=zf, in0=sf, scalar1=0.5, scalar2=None, op0=A.is_lt)
            lf128 = main.tile([P, 1], f32)
            nc.vector.tensor_scalar(out=lf128, in0=lf, scalar1=128.0, scalar2=None, op0=A.mult)
            pk0 = main.tile([P, 1], f32)
            nc.vector.tensor_scalar(
                out=pk0, in0=sf, scalar1=16384.0, scalar2=lf128[:, 0:1],
                op0=A.mult, op1=A.add,
            )
            pkf = main.tile([P, 1], f32)
            nc.vector.tensor_scalar(
                out=pkf, in0=zf, scalar1=2097152.0, scalar2=pk0[:, 0:1],
                op0=A.mult, op1=A.add,
            )

            # ---- broadcast pack column via stride-0 transpose ----
            pb_ps = psum_pb.tile([P, V], f32)
            nc.tensor.transpose(pb_ps, bcast_last(pkf[:, 0:1], V), ident)
            nc.vector.tensor_copy(sel[:, 0:V], pb_ps)

            # ---- per-slot fused dot: comb = pack[i0] + a1 ----
            comb = main.tile([P, NJ], f32)
            for j in range(NJ):
                scr = scrp.tile([P, 2 * V], f32, tag=f"s{j % 2}")
                nc.vector.scalar_tensor_tensor(
                    out=scr, in0=xt[:, j, :], scalar=1.0, in1=sel,
                    op0=A.mult, op1=A.mult, accum_out=comb[:, j : j + 1],
                )

            # ---- unpack comb -> compare index ----
            ci = main.tile([P, NJ], i32)
            nc.vector.tensor_copy(ci, comb)
            a1i = main.tile([P, NJ], i32)
            nc.vector.tensor_scalar(out=a1i, in0=ci, scalar1=127, scalar2=None, op0=A.bitwise_and)
            s14 = main.tile([P, NJ], i32)
            nc.vector.tensor_scalar(out=s14, in0=ci, scalar1=14, scalar2=None, op0=A.arith_shift_right)
            s7 = main.tile([P, NJ], i32)
            nc.vector.tensor_scalar(out=s7, in0=ci, scalar1=7, scalar2=None, op0=A.arith_shift_right)
            z8 = main.tile([P, NJ], i32)
            nc.vector.tensor_scalar(
                out=z8, in0=ci, scalar1=21, scalar2=8,
                op0=A.arith_shift_right, op1=A.arith_shift_left,
            )
            ti = main.tile([P, NJ], i32)
            nc.vector.tensor_mul(ti, s14, a1i)
            t2 = main.tile([P, NJ], i32)
            nc.vector.tensor_add(t2, s7, ti)
            c0 = main.tile([P, NJ], i32)
            nc.vector.tensor_scalar(out=c0, in0=t2, scalar1=127, scalar2=None, op0=A.bitwise_and)
            cr = main.tile([P, NJ], i32)
            nc.vector.tensor_tensor(out=cr, in0=c0, in1=z8, op=A.bitwise_or)
            cf = main.tile([P, NJ], f32)
            nc.vector.tensor_copy(cf, cr)

            # ---- z1 in place; stream halves out ----
            for h in range(2):
                js = h * HJ
                nc.vector.tensor_tensor(
                    out=xt[:, js : js + HJ, V : 2 * V],
                    in0=bcast_mid(iota_f, HJ),
                    in1=bcast_last(cf[:, js : js + HJ], V),
                    op=A.is_equal,
                )
                nc.sync.dma_start(
                    out=out_r[:, js : js + HJ, :], in_=xt[:, js : js + HJ, :]
                )

    nc.finalize()
    return nc


def _split_bf16(a: np.ndarray):
    hi = a.astype(ml_dtypes.bfloat16)
    lo = (a - hi.astype(np.float32)).astype(ml_dtypes.bfloat16)
    return hi, lo


def _host_marshal(W1, b1, W2, b2):
    w1tb = np.empty((P, KH, V + 1), np.float32)
    w1tb[:, :, :V] = W1.T.reshape(P, KH, V)
    w1tb[:, :, V] = b1.reshape(P, KH)
    w2hv, w2lv = _split_bf16(W2.astype(np.float32))
    b2h, b2l = _split_bf16(b2.astype(np.float32))
    w2ha = np.ascontiguousarray(w2hv.reshape(P, KH, 2 * V))
    w2lb = np.zeros((P, KH * 2 * V + 2 * V), ml_dtypes.bfloat16)
    w2lb[:, 0 : KH * 2 * V] = w2lv.reshape(P, KH * 2 * V)
    w2lb[0, KH * 2 * V :] = b2h
    w2lb[1, KH * 2 * V :] = b2l
    return np.ascontiguousarray(w1tb), w2ha, np.ascontiguousarray(w2lb)


RUN_KWARGS: dict = {}
LAST_RESULTS = None


def kernel(**inputs) -> np.ndarray:
    global LAST_RESULTS
    from concourse.bass_utils import run_bass_kernel_spmd

    x = np.ascontiguousarray(np.asarray(inputs["inputs"], dtype=np.float32))
    W1 = np.asarray(inputs["W1"], dtype=np.float32)
    b1 = np.asarray(inputs["b1"], dtype=np.float32)
    W2 = np.asarray(inputs["W2"], dtype=np.float32)
    b2 = np.asarray(inputs["b2"], dtype=np.float32)
    w1tb, w2ha, w2lb = _host_marshal(W1, b1, W2, b2)

    B = x.shape[0]
    rows = B // N_CORES
    nc = build_bass(rows)

    shards = np.split(x, N_CORES, axis=0)
    in_maps = [{"x": s, "w1tb": w1tb, "w2h": w2ha, "w2lb": w2lb} for s in shards]
    res = run_bass_kernel_spmd(nc, in_maps, list(range(N_CORES)), **RUN_KWARGS)
    LAST_RESULTS = res
    return np.concatenate([r["out"] for r in res.results], axis=0)
